# revision 1
# baseline (speedup 1.0000x reference)
"""Trainium2 Bass kernel for batched multi-head self-attention.

Reference computation (per batch element b):
    qkv = x @ w_qkv.T                  # [N, 3C]
    q, k, v = split/reshape to heads   # H=16 heads, d=64
    attn = softmax(q @ k.T / sqrt(d))
    out = (attn @ v) reshaped back     # [N, C]
    y = out @ w_proj.T + b_proj

Sharding: pure data-parallel over batch B=8 across the 8 NeuronCores
(one batch element per core, weights replicated, no collectives).

On-device layout (everything transposed so matmuls contract over the
partition axis with no on-device transposes):
  - xT      [C, N]   (host pre-transposed, bf16)
  - wqkvT   [C, 3C]  (host pre-transposed, bf16)
  - wprojT  [C, C]   (host pre-transposed, bf16)
  - scores computed as S^T tiles [m, n]; softmax row-sums obtained by
    appending a ones-column to V in the attn@V matmul (PE computes the
    sums for free); normalization applied at PSUM evacuation.

Performance structure (TimelineSim: 229.2us, PE 96.5% busy; the
all-matmul floor is 218.4us = 1024 MMs x 213ns):
  - bf16 matmuls everywhere (fp32 is 4x slower on the PE); fp32 PSUM
    accumulation and fp32 softmax scores keep rel err ~6e-3.
  - QK^T head pairs are row-packed via tile_position (K=64 each, rows
    0-63 / 64-127) -- concurrent on silicon.
  - Emission interleaves attention (ACT-heavy) with the q/k projection
    (PE-heavy) per head-pair so the scheduler fills softmax-bound PE
    bubbles with projection matmuls; output projection is emitted
    n2-outer to fill the last pair's tail.
  - PSUM budget (8 banks): acc 3 + st 3 + av 2; attention loops
    n2-outer so only one n2's AV accumulators are live. Phases whose
    natural tags are idle borrow the other tags' slots (v-projection
    rotates across all three; the final projection chains spread over
    all 8 slots so their preludes run before the last pair lands).
  - Dummy warm-up matmuls during the initial DMA wait complete the
    PE p-state/HAM ramp before real work arrives.
  - x^T and the v-columns of w_qkv are fused host-side into one "xw"
    tensor: one DMA per contraction tile (DMA-start overhead is a
    fixed cost per descriptor).
  - Output stored bf16 (halves store transfers incl. the tail-critical
    one); host converts back to f32. Adds ~0.2% RMS quantization --
    total rel err 6.0e-3 vs the 2e-2 gate.
"""

import os
import sys

for _p in ("/opt/trn_rl_repo", "/root/.axon_site/_ro/trn_rl_repo"):
    if os.path.isdir(_p) and _p not in sys.path:
        sys.path.insert(0, _p)
        break

import numpy as np
import ml_dtypes

import concourse.bass as bass
import concourse.bacc as bacc
import concourse.tile as tile
import concourse.mybir as mybir
from concourse import bass_utils

BF16 = mybir.dt.bfloat16
F32 = mybir.dt.float32
AF = mybir.ActivationFunctionType

B, N, C, H = 8, 1024, 1024, 16
D = C // H            # 64 head dim
P = 128               # partitions
CT = C // P           # 8 contraction tiles
NT2 = N // 512        # 2 n-tiles of 512
MT = N // P           # 8 m-tiles of 128
PAIRS = H // 2        # 8 head pairs
SCALE = float(D) ** -0.5
N_CORES = 8

_cache = {}


def _build():
    nc = bacc.Bacc("TRN2", target_bir_lowering=False, debug=False,
                   enable_asserts=False, num_devices=N_CORES)

    xw_d = nc.dram_tensor("xw", [C, 2 * N], BF16, kind="ExternalInput")
    wqkvT_d = nc.dram_tensor("wqkvT", [C, 3 * C], BF16, kind="ExternalInput")
    wprojT_d = nc.dram_tensor("wprojT", [C, C], BF16, kind="ExternalInput")
    bias_d = nc.dram_tensor("bias", [P, CT], F32, kind="ExternalInput")
    outT_d = nc.dram_tensor("outT", [C, N], BF16, kind="ExternalOutput")

    with tile.TileContext(nc) as tc:
        with (
            tc.tile_pool(name="res", bufs=1) as rp,
            tc.tile_pool(name="work", bufs=2) as wp,
            tc.tile_pool(name="ps", bufs=1, space="PSUM") as pp,
        ):
            # ---------------- PE warm-up ----------------
            # The PE sits idle ~3.5us waiting for the first input DMAs, and
            # the p-state/HAM ramp then penalizes the first ~3us of real
            # matmuls. Run dummy matmuls on memset data during the DMA wait
            # so the ramp completes before real work arrives.
            warm_a = wp.tile([P, 512], BF16, name="warm_a", tag="warm_a",
                             bufs=1)
            nc.gpsimd.memset(warm_a[:], 0.25)
            warm_ps = pp.tile([P, 512], F32, name="warm_ps", tag="acc",
                              bufs=3)
            for _ in range(6):
                nc.tensor.matmul(warm_ps[:], warm_a[:, 0:P], warm_a[:],
                                 start=True, stop=True)

            # ---------------- resident inputs ----------------
            # v-part inputs first so the first matmuls can start ASAP.
            # x and the v-columns of w_qkv are fused host-side into one
            # tensor: one DMA per c-tile (the HWDGE queue stage costs a
            # fixed ~625ns per DMA, so fewer DMAs = faster start)
            xT = []
            wqv = []
            for i in range(CT):
                t = rp.tile([P, 2 * N], BF16, name=f"xw{i}", tag=f"xw{i}")
                nc.sync.dma_start(t[:], xw_d.ap()[i * P:(i + 1) * P, :])
                xT.append(t[:, 0:N])
                wqv.append(t[:, N:2 * N])
            wqk = []
            for i in range(CT):
                t = rp.tile([P, 2 * C], BF16, name=f"wqk{i}", tag=f"wqk{i}")
                nc.sync.dma_start(t[:],
                                  wqkvT_d.ap()[i * P:(i + 1) * P, 0:2 * C])
                wqk.append(t)
            wpj = []
            for i in range(CT):
                t = rp.tile([P, C], BF16, name=f"wpj{i}", tag=f"wpj{i}")
                nc.sync.dma_start(t[:], wprojT_d.ap()[i * P:(i + 1) * P, :])
                wpj.append(t)
            bias_t = rp.tile([P, CT], F32, name="bias_t", tag="bias")
            nc.sync.dma_start(bias_t[:], bias_d.ap())

            # ---------------- result tiles ----------------
            qT = [rp.tile([P, N], BF16, name=f"qT{i}", tag=f"qT{i}")
                  for i in range(PAIRS)]
            kT = [rp.tile([P, N], BF16, name=f"kT{i}", tag=f"kT{i}")
                  for i in range(PAIRS)]
            vt = [[rp.tile([P, 8, D + 1], BF16, name=f"v{m}_{j}",
                           tag=f"v{m}_{j}") for j in range(2)]
                  for m in range(MT)]
            ao = [rp.tile([P, N], BF16, name=f"ao{i}", tag=f"ao{i}")
                  for i in range(PAIRS)]

            for m in range(MT):
                for j in range(2):
                    nc.vector.memset(vt[m][j][:, :, D:D + 1], 1.0)

            # ---------------- phase A(v): v projection ----------------
            # Alternate psum tags: the attention-phase "av" slots are idle
            # here, so borrow them for 4-deep accumulator pipelining.
            for m in range(MT):
                for j in range(2):
                    vtag, vbufs = (("acc", 3), ("av", 2),
                                   ("st", 3))[(2 * m + j) % 3]
                    ps = pp.tile([P, 512], F32, name=f"accv{m}_{j}", tag=vtag,
                                 bufs=vbufs)
                    for c in range(CT):
                        nc.tensor.matmul(
                            ps[:],
                            xT[c][:, m * P:(m + 1) * P],
                            wqv[c][:, j * 512:(j + 1) * 512],
                            start=(c == 0), stop=(c == CT - 1),
                        )
                    nc.vector.tensor_copy(
                        vt[m][j][:, :, 0:D],
                        ps[:].rearrange("p (h d) -> p h d", d=D),
                    )

            # ------- interleaved: q/k projection + attention per pair -------
            for pr in range(PAIRS):
                for which, dst in ((0, qT[pr]), (1, kT[pr])):
                    o0 = which * C + pr * P
                    for n2 in range(NT2):
                        nsl = slice(n2 * 512, (n2 + 1) * 512)
                        ps = pp.tile([P, 512], F32,
                                     name=f"accqk{pr}_{which}_{n2}",
                                     tag="acc", bufs=3)
                        for c in range(CT):
                            nc.tensor.matmul(
                                ps[:],
                                wqk[c][:, o0:o0 + P],
                                xT[c][:, nsl],
                                start=(c == 0), stop=(c == CT - 1),
                            )
                        nc.vector.tensor_copy(dst[:, nsl], ps[:])

                # attention for this pair (n2-outer so only one n2's AV
                # accumulators are live; PSUM budget: acc 3 + st 3 + av 2 = 8)
                for n2 in range(NT2):
                    nsl = slice(n2 * 512, (n2 + 1) * 512)
                    av = [pp.tile([D + 1, 512], F32, name=f"av{pr}_{n2}_{h}",
                                  tag="av", bufs=2) for h in range(2)]
                    for m in range(MT):
                        msl = slice(m * P, (m + 1) * P)
                        st = [pp.tile([P, 512], F32,
                                      name=f"st{pr}_{m}_{n2}_{h}", tag="st",
                                      bufs=3) for h in range(2)]
                        for h in range(2):
                            psl = slice(h * 64, (h + 1) * 64)
                            nc.tensor.matmul(
                                st[h][:],
                                kT[pr][psl, msl],
                                qT[pr][psl, nsl],
                                start=True, stop=True,
                                tile_position=(h * 64, 0),
                            )
                        for h in range(2):
                            pt = wp.tile([P, 512], BF16,
                                         name=f"pt{pr}_{m}_{n2}_{h}",
                                         tag="pt", bufs=6)
                            nc.scalar.activation(pt[:], st[h][:], AF.Exp,
                                                 scale=SCALE)
                            head = 2 * pr + h
                            vtile = vt[m][head // 8]
                            nc.tensor.matmul(
                                av[h][:],
                                vtile[:, head % 8, :],
                                pt[:],
                                start=(m == 0), stop=(m == MT - 1),
                            )
                    # normalize + evacuate this n2 slice. Copy PSUM out
                    # first so the av bank frees fast; normalize from SBUF.
                    # For the very last slice the bank release doesn't
                    # matter; read PSUM directly to shorten the tail chain.
                    last_slice = (pr == PAIRS - 1 and n2 == NT2 - 1)
                    for h in range(2):
                        if last_slice:
                            araw = av[h]
                        else:
                            araw = wp.tile([D + 1, 512], F32,
                                           name=f"araw{pr}_{h}_{n2}",
                                           tag="araw", bufs=4)
                            nc.vector.tensor_copy(araw[:], av[h][:])
                        rec = wp.tile([D + 1, 512], F32,
                                      name=f"rec{pr}_{h}_{n2}", tag="rec",
                                      bufs=4)
                        nc.vector.reciprocal(rec[D:D + 1, :],
                                             araw[D:D + 1, :])
                        rec0 = wp.tile([1, 512], F32,
                                       name=f"rec0_{pr}_{h}_{n2}", tag="rec0",
                                       bufs=4)
                        nc.sync.dma_start(rec0[:], rec[D:D + 1, :])
                        bc = wp.tile([D, 512], F32, name=f"bc{pr}_{h}_{n2}",
                                     tag="bc", bufs=4)
                        nc.gpsimd.partition_broadcast(bc[:], rec0[:])
                        if h == 0:
                            nc.vector.tensor_mul(ao[pr][0:D, nsl],
                                                 araw[0:D, :], bc[:])
                        else:
                            tmp = wp.tile([D, 512], BF16,
                                          name=f"aotmp{pr}_{n2}", tag="aotmp",
                                          bufs=4)
                            nc.vector.tensor_mul(tmp[:], araw[0:D, :],
                                                 bc[:])
                            nc.sync.dma_start(ao[pr][D:P, nsl], tmp[:])

            # ---------------- phase C: output projection ----------------
            # n2-outer: proj over n2=0 becomes ready while the last pair's
            # n2=1 attention still runs, filling the PE tail gap.
            for n2 in range(NT2):
                for ot in range(CT):
                    nsl = slice(n2 * 512, (n2 + 1) * 512)
                    # n2=1 runs at the very end when the attention's st/av
                    # slots are dead: spread the 8 chains across all tags so
                    # every pair-0..6 prelude can run before ao[7] arrives
                    if n2 == 0:
                        ptag, pbufs = "acc", 3
                    else:
                        ptag, pbufs = (("acc", 3), ("st", 3), ("av", 2),
                                       ("acc", 3), ("st", 3), ("av", 2),
                                       ("acc", 3), ("st", 3))[ot]
                    ps = pp.tile([P, 512], F32, name=f"accy{ot}_{n2}",
                                 tag=ptag, bufs=pbufs)
                    for pr in range(PAIRS):
                        nc.tensor.matmul(
                            ps[:],
                            wpj[pr][:, ot * P:(ot + 1) * P],
                            ao[pr][:, nsl],
                            start=(pr == 0), stop=(pr == PAIRS - 1),
                        )
                    yt = wp.tile([P, 512], BF16, name=f"y{ot}_{n2}", tag="y",
                                 bufs=3)
                    nc.vector.tensor_scalar_add(yt[:], ps[:],
                                                bias_t[:, ot:ot + 1])
                    nc.sync.dma_start(outT_d.ap()[ot * P:(ot + 1) * P, nsl],
                                      yt[:])

    nc.compile()
    return nc


def get_nc():
    if "nc" not in _cache:
        _cache["nc"] = _build()
    return _cache["nc"]


def kernel(x, w_qkv, w_proj, b_proj):
    x = np.asarray(x, dtype=np.float32)
    w_qkv = np.asarray(w_qkv, dtype=np.float32)
    w_proj = np.asarray(w_proj, dtype=np.float32)
    b_proj = np.asarray(b_proj, dtype=np.float32)

    bf = ml_dtypes.bfloat16
    wqkvT = np.ascontiguousarray(w_qkv.T).astype(bf)     # [C, 3C]
    wprojT = np.ascontiguousarray(w_proj.T).astype(bf)   # [C, C]
    bias = np.ascontiguousarray(b_proj.reshape(CT, P).T).astype(np.float32)

    in_maps = []
    wqv_host = wqkvT[:, 2 * C:]                          # [C, C] v columns
    for b in range(N_CORES):
        xT = np.ascontiguousarray(x[b].T).astype(bf)     # [C, N]
        xw = np.ascontiguousarray(np.concatenate([xT, wqv_host], axis=1))
        in_maps.append({"xw": xw, "wqkvT": wqkvT, "wprojT": wprojT,
                        "bias": bias})

    nc = get_nc()
    _cache["in_maps"] = in_maps
    res = bass_utils.run_bass_kernel_spmd(nc, in_maps,
                                          core_ids=list(range(N_CORES)))
    out = np.empty((B, N, C), dtype=np.float32)
    for b in range(N_CORES):
        out[b] = res.results[b]["outT"].T.astype(np.float32)
    return out



# revision 60
# speedup vs baseline: 1.1759x; 1.1759x over previous
"""Trainium2 Bass kernel for batched multi-head self-attention.

Reference computation (per batch element b):
    qkv = x @ w_qkv.T                  # [N, 3C]
    q, k, v = split/reshape to heads   # H=16 heads, d=64
    attn = softmax(q @ k.T / sqrt(d))
    out = (attn @ v) reshaped back     # [N, C]
    y = out @ w_proj.T + b_proj

Sharding: pure data-parallel over batch B=8 across the 8 NeuronCores
(one batch element per core, weights replicated, no collectives).

Performance structure (vs the 229us all-bf16 version):
  - All three projections (qkv, out) run as residual-corrected fp8:
    operands split hi (e4m3) + lo (e5m2, captures the e4m3 rounding
    residue; e5m2's wider exponent range keeps the small residues from
    underflowing).  y = x_hi.w_hi + x_hi.w_lo + x_lo.w_hi, the dropped
    lo.lo term is ~0.1%.  DoubleRow perf-mode matmuls contract 2 k-tiles
    (K=256) per instruction at 0.5 cycles/row: a K=1024 projection is
    4 main + 8 cross = 12 instructions vs 8 bf16 ones, at ~107ns each
    vs 213ns -> 0.60x the PE time with near-bf16 accuracy.
    Host pre-splits x and the weights; pair-grouped layouts
    [128, g, 2, cols] put both k-tiles of a DoubleRow instruction in
    the same partitions. Cross terms pair group g of one operand's hi
    with group g of the other's lo -- same logical c either way.
  - Scores stay bf16 (K=64 per head: DoubleRow can't drop below one
    instruction per psum tile, so fp8 wouldn't save anything); head
    pairs are row-packed via tile_position as before.
  - attn@V runs transposed ("AV-T"): stationary = pt tile [keys, q128],
    moving = v [keys, 65] (64 dims + ones column for the softmax sums).
    Streaming 65 columns instead of 512 cuts AV PE time 4x; the cost
    model charges out-free-size cycles and weight loads are free.
    Output lands [q, d]: softmax normalization becomes a per-partition
    scalar multiply (reciprocal of the ones-column sum), then [128,128]
    blocks (2 heads x 64 dims) are transposed back on the PE against a
    host-provided identity and split hi/lo into the fp8 operands of the
    output projection.
  - PSUM: acc 2 banks + st 2x2 + av/tr 2 = 8. Multi-chunk tiles (4
    AV-T accumulators per bank, 4 transposes per bank) share a single
    accumulation group: start=True on the bank's first write zeroes the
    whole 2KB zero-region, later disjoint writes use start=False.
  - Scores produce into a 2-bank [128, 2, 512] tile so one ACT exp
    instruction covers both heads (halves ACT instruction overhead).
  - Emission interleaves projection chains (a filler queue) into the
    score/exp pipeline bubbles; out-proj for the first n-half fills the
    last pair's second-half attention, warm-up matmuls cover the
    initial DMA wait and PE p-state ramp.
"""

import os
import sys

for _p in ("/opt/trn_rl_repo", "/root/.axon_site/_ro/trn_rl_repo"):
    if os.path.isdir(_p) and _p not in sys.path:
        sys.path.insert(0, _p)
        break

from collections import deque

import numpy as np
import ml_dtypes

import concourse.bass as bass
import concourse.bacc as bacc
import concourse.tile as tile
import concourse.mybir as mybir
from concourse import bass_utils

BF16 = mybir.dt.bfloat16
F32 = mybir.dt.float32
FP8E4 = mybir.dt.float8e4
FP8E5 = mybir.dt.float8e5
DR = mybir.MatmulPerfMode.DoubleRow
AF = mybir.ActivationFunctionType
E4 = ml_dtypes.float8_e4m3
E5 = ml_dtypes.float8_e5m2

B, N, C, H = 8, 1024, 1024, 16
D = C // H            # 64 head dim
P = 128               # partitions
NT2 = N // 512        # 2 n-tiles of 512
MT = N // P           # 8 m-tiles (key tiles) of 128
PAIRS = H // 2        # 8 head pairs
QC = 4                # 128-wide q chunks per 512 n-half
SCALE = float(D) ** -0.5
N_CORES = 8

_cache = {}


def _build():
    nc = bacc.Bacc("TRN2", target_bir_lowering=False, debug=False,
                   enable_asserts=False, num_devices=N_CORES)

    x4_d = nc.dram_tensor("x4", [P, 8, N], FP8E4, kind="ExternalInput")
    x5_d = nc.dram_tensor("x5", [P, 8, N], FP8E5, kind="ExternalInput")
    w4_d = nc.dram_tensor("w4", [P, 8, 3 * C], FP8E4, kind="ExternalInput")
    w5_d = nc.dram_tensor("w5", [P, 8, 3 * C], FP8E5, kind="ExternalInput")
    wp4_d = nc.dram_tensor("wp4", [P, 8, C], FP8E4, kind="ExternalInput")
    wp5_d = nc.dram_tensor("wp5", [P, 8, C], FP8E5, kind="ExternalInput")
    bias_d = nc.dram_tensor("bias", [P, MT], F32, kind="ExternalInput")
    id_d = nc.dram_tensor("ident", [P, P], BF16, kind="ExternalInput")
    outT_d = nc.dram_tensor("outT", [C, N], BF16, kind="ExternalOutput")
    dbg_d = nc.dram_tensor("dbg", [P, 8, N], FP8E4, kind="ExternalOutput")
    dbgq_d = nc.dram_tensor("dbgq", [P, N], BF16, kind="ExternalOutput")
    dbgp_d = nc.dram_tensor("dbgp", [P, 2, 512], BF16, kind="ExternalOutput")

    with tile.TileContext(nc) as tc:
        with (
            tc.tile_pool(name="res", bufs=1) as rp,
            tc.tile_pool(name="ps", bufs=1, space="PSUM") as pp,
        ):
            # ---------------- PE warm-up ----------------
            warm_a = rp.tile([P, 512], BF16, name="warm_a", tag="warm_a")
            nc.gpsimd.memset(warm_a[:], 0.25)
            warm_ps = pp.tile([P, 512], F32, name="warm_ps", tag="acc",
                              bufs=2)
            for _ in range(10):
                nc.tensor.matmul(warm_ps[:], warm_a[:, 0:P], warm_a[:],
                                 start=True, stop=True)

            # ---------------- resident inputs ----------------
            x4t = rp.tile([P, 8, N], FP8E4, name="x4t", tag="x4t")
            x5t = rp.tile([P, 8, N], FP8E5, name="x5t", tag="x5t")
            w4t = rp.tile([P, 8, 3 * C], FP8E4, name="w4t", tag="w4t")
            w5t = rp.tile([P, 8, 3 * C], FP8E5, name="w5t", tag="w5t")
            wp4t = rp.tile([P, 8, C], FP8E4, name="wp4t", tag="wp4t")
            wp5t = rp.tile([P, 8, C], FP8E5, name="wp5t", tag="wp5t")
            # Two HWDGE queues exist (SP via nc.sync, ACT via nc.scalar)
            # and each DMA pays a fixed ~625ns queue stage, so batch
            # transfers and split them across both queues in first-use
            # order: pair0's q/k weight columns, n2=0 x halves, v
            # weights, remaining x, remaining q/k columns, w_proj.
            h0, h1 = slice(0, 512), slice(512, N)
            q0, k0 = slice(0, P), slice(C, C + P)
            qr, kr = slice(P, C), slice(C + P, 2 * C)
            vs = slice(2 * C, 3 * C)
            ident = rp.tile([P, P], BF16, name="ident", tag="ident")
            bias_t = rp.tile([P, MT], F32, name="bias_t", tag="bias")
            # The ACT queue only carries triggers needed before its exp
            # stream starts -- later triggers would block the exps
            # in-queue while waiting for the shared transfer engine.
            sp = nc.sync
            sp.dma_start(x4t[:, :, h0], x4_d.ap()[:, :, h0])
            sp.dma_start(x5t[:, :, h0], x5_d.ap()[:, :, h0])
            sp.dma_start(w4t[:, :, q0], w4_d.ap()[:, :, q0])
            sp.dma_start(w4t[:, :, k0], w4_d.ap()[:, :, k0])
            sp.dma_start(w5t[:, :, q0], w5_d.ap()[:, :, q0])
            sp.dma_start(w5t[:, :, k0], w5_d.ap()[:, :, k0])
            sp.dma_start(x4t[:, :, h1], x4_d.ap()[:, :, h1])
            sp.dma_start(x5t[:, :, h1], x5_d.ap()[:, :, h1])
            sp.dma_start(ident[:], id_d.ap())
            sp.dma_start(bias_t[:], bias_d.ap())
            sp.dma_start(w4t[:, :, vs], w4_d.ap()[:, :, vs])
            sp.dma_start(w5t[:, :, vs], w5_d.ap()[:, :, vs])
            sp.dma_start(w4t[:, :, qr], w4_d.ap()[:, :, qr])
            sp.dma_start(w5t[:, :, qr], w5_d.ap()[:, :, qr])
            sp.dma_start(w4t[:, :, kr], w4_d.ap()[:, :, kr])
            sp.dma_start(w5t[:, :, kr], w5_d.ap()[:, :, kr])
            sp.dma_start(wp4t[:], wp4_d.ap())
            sp.dma_start(wp5t[:], wp5_d.ap())

            # ---------------- result tiles ----------------
            qT = [rp.tile([P, N], BF16, name=f"qT{i}", tag=f"qT{i}")
                  for i in range(PAIRS)]
            kT = [rp.tile([P, N], BF16, name=f"kT{i}", tag=f"kT{i}")
                  for i in range(PAIRS)]
            vt = [[rp.tile([P, 8, D + 1], BF16, name=f"v{m}_{j}",
                           tag=f"v{m}_{j}") for j in range(2)]
                  for m in range(MT)]
            ao4t = rp.tile([P, 8, N], FP8E4, name="ao4t", tag="ao4t")
            ao5t = rp.tile([P, 8, N], FP8E5, name="ao5t", tag="ao5t")

            for m in range(MT):
                for j in range(2):
                    nc.vector.memset(vt[m][j][:, :, D:D + 1], 1.0)

            # -------- corrected-fp8 chain emitters --------
            def fp8_chain(ps, lhs4, lhs5, rhs4, rhs5, last_g=None):
                """main hi.hi (4 DR) + cross hi.lo / lo.hi (8 DR).
                last_g: emit that group's three products last so a chain
                whose final input group lands late can start early."""
                order = list(range(4))
                if last_g is not None:
                    order.remove(last_g)
                    order.append(last_g)
                steps = [(lhs4, rhs4, g) for g in order]                     + [(lhs5, rhs4, g) for g in order]                     + [(lhs4, rhs5, g) for g in order]
                if last_g is not None:
                    steps.sort(key=lambda t: t[2] == last_g)
                for i, (lf, rf, g) in enumerate(steps):
                    nc.tensor.matmul(ps, lf(g), rf(g),
                                     start=(i == 0), stop=(i == 11),
                                     perf_mode=DR)

            def v_chain(m, j):
                msl = slice(m * P, (m + 1) * P)
                vsl = slice(2 * C + j * 512, 2 * C + (j + 1) * 512)
                ps = pp.tile([P, 512], F32, name=f"accv{m}_{j}", tag="acc",
                             bufs=2)
                fp8_chain(ps[:],
                          lambda g: x4t[:, 2 * g:2 * g + 2, msl],
                          lambda g: x5t[:, 2 * g:2 * g + 2, msl],
                          lambda g: w4t[:, 2 * g:2 * g + 2, vsl],
                          lambda g: w5t[:, 2 * g:2 * g + 2, vsl])
                nc.vector.tensor_copy(
                    vt[m][j][:, :, 0:D],
                    ps[:].rearrange("p (h d) -> p h d", d=D))

            def qk_chain(pr, which, n2):
                o0 = which * C + pr * P
                osl = slice(o0, o0 + P)
                nsl = slice(n2 * 512, (n2 + 1) * 512)
                ps = pp.tile([P, 512], F32, name=f"accqk{pr}_{which}_{n2}",
                             tag="acc", bufs=2)
                fp8_chain(ps[:],
                          lambda g: w4t[:, 2 * g:2 * g + 2, osl],
                          lambda g: w5t[:, 2 * g:2 * g + 2, osl],
                          lambda g: x4t[:, 2 * g:2 * g + 2, nsl],
                          lambda g: x5t[:, 2 * g:2 * g + 2, nsl])
                dst = qT[pr] if which == 0 else kT[pr]
                nc.vector.tensor_copy(dst[:, nsl], ps[:])

            def proj_chain(n2, ot):
                osl = slice(ot * P, (ot + 1) * P)
                nsl = slice(n2 * 512, (n2 + 1) * 512)
                ps = pp.tile([P, 512], F32, name=f"accy{ot}_{n2}", tag="acc",
                             bufs=2)
                fp8_chain(ps[:],
                          lambda g: wp4t[:, 2 * g:2 * g + 2, osl],
                          lambda g: wp5t[:, 2 * g:2 * g + 2, osl],
                          lambda g: ao4t[:, 2 * g:2 * g + 2, nsl],
                          lambda g: ao5t[:, 2 * g:2 * g + 2, nsl],
                          last_g=3)
                yt = rp.tile([P, 512], BF16, name=f"y{ot}_{n2}", tag="y",
                             bufs=3)
                nc.vector.tensor_scalar_add(yt[:], ps[:],
                                            bias_t[:, ot:ot + 1])
                eng = nc.scalar if (n2 == 1 and ot % 2 == 1) else nc.sync
                eng.dma_start(outT_d.ap()[osl, nsl], yt[:])

            # Filler queue: independent PE chains interleaved into the
            # attention pipeline's exp-wait bubbles. Due-date keys are
            # 2*block + phase with block = 2*pr + n2 and phase 0 = block
            # top (scores need it), 1 = before that block's AV-T.
            # Pairs 0-3 only read the j=0 half of v (heads 0-7), so the
            # j=1 v chains are due by pair 4; j=0 fills pair 0's wait
            # for its own exp stream.
            fillers = deque()

            def add_filler(key, fn):
                assert not fillers or fillers[-1][0] <= key, \
                    "filler due-keys must be nondecreasing"
                fillers.append((key, fn))

            for m in range(MT):
                add_filler(1, lambda m=m: v_chain(m, 0))
            add_filler(2, lambda: qk_chain(0, 0, 1))
            # deque keys MUST be nondecreasing or drain() stalls behind a
            # later-due entry and starves an earlier-due one
            for pr in range(1, 5):
                for which in range(2):
                    for n2 in range(NT2):
                        add_filler(
                            4 * pr,
                            lambda pr=pr, w=which, n2=n2: qk_chain(pr, w, n2))
            for m in range(MT):
                add_filler(17, lambda m=m: v_chain(m, 1))
            for pr in range(5, PAIRS):
                for which in range(2):
                    for n2 in range(NT2):
                        add_filler(
                            4 * pr,
                            lambda pr=pr, w=which, n2=n2: qk_chain(pr, w, n2))

            def drain(key):
                while fillers and fillers[0][0] <= key:
                    fillers.popleft()[1]()

            def pop_filler(blk):
                # only pop work due within the next couple of blocks, so
                # late-due chains are held back for the bubble that needs
                # them instead of being consumed eagerly
                if fillers and fillers[0][0] <= 2 * blk + 4:
                    fillers.popleft()[1]()

            # pair 0 up front: q first half, k BOTH halves (scores of
            # either n-half contract over the full key range of kT)
            qk_chain(0, 0, 0)
            qk_chain(0, 1, 0)
            qk_chain(0, 1, 1)

            # Deferred back-end of a block: transposes + hi/lo splits.
            # Emitted at the NEXT block's top so the PE does them while
            # that block's exp stream warms up, instead of stalling on
            # the normalize chain at this block's end.
            pending = []

            def flush_pending():
                while pending:
                    aobs, fpr, fn2 = pending.pop(0)
                    tr = pp.tile([P, QC, P], BF16, name=f"tr{fpr}_{fn2}",
                                 tag="av", bufs=2)
                    for qc in range(QC):
                        nc.tensor.matmul(tr[:, qc, :], aobs[qc][:], ident[:],
                                         is_transpose=True,
                                         start=(qc == 0), stop=(qc == QC - 1),
                                         skip_group_check=True)
                    # single full-tile evacuation: its read covers the
                    # whole bank, so a later start=True bank-zero there
                    # cannot race an unread chunk (WAR deps are byte-
                    # range based but the zero region is the full bank)
                    aob2 = rp.tile([P, QC, P], BF16, name=f"aob2{fpr}_{fn2}",
                                   tag="aob2", bufs=2)
                    nc.vector.tensor_copy(aob2[:], tr[:])
                    for qc in range(QC):
                        qsl = slice(fn2 * 512 + qc * P,
                                    fn2 * 512 + (qc + 1) * P)
                        hi = ao4t[:, fpr, qsl]
                        nc.gpsimd.tensor_copy(hi, aob2[:, qc, :])
                        nc.gpsimd.tensor_sub(ao5t[:, fpr, qsl],
                                             aob2[:, qc, :], hi)

            # ---------------- attention pair loop ----------------
            for pr in range(PAIRS):
                for n2 in range(NT2):
                    blk = 2 * pr + n2
                    drain(2 * blk)
                    flush_pending()
                    nsl0 = n2 * 512
                    # scores + fused exp per key tile
                    pts = [rp.tile([P, 2, 512], BF16,
                                   name=f"pt{pr}_{n2}_{m}", tag=f"pt{m}",
                                   bufs=2) for m in range(MT)]
                    for m in range(MT):
                        msl = slice(m * P, (m + 1) * P)
                        st2 = pp.tile([P, 2, 512], F32,
                                      name=f"st{pr}_{n2}_{m}", tag="st",
                                      bufs=2)
                        for h in range(2):
                            psl = slice(h * 64, (h + 1) * 64)
                            nc.tensor.matmul(
                                st2[:, h, :],
                                kT[pr][psl, msl],
                                qT[pr][psl, nsl0:nsl0 + 512],
                                start=True, stop=True,
                                tile_position=(h * 64, 0))
                        nc.scalar.activation(pts[m][:], st2[:], AF.Exp,
                                             scale=SCALE)
                        if (m in (3, 6)) or blk == 2 * PAIRS - 1:
                            pop_filler(99 if blk == 2 * PAIRS - 1 else blk)
                    # AV-T: accumulate over key tiles, 4 q-chunks per bank
                    drain(2 * blk + 1)
                    av4 = [pp.tile([P, QC, D + 1], F32,
                                   name=f"av{pr}_{n2}_{h}", tag="av",
                                   bufs=2) for h in range(2)]
                    for m in range(MT):
                        if m == 5 or (blk == 2 * PAIRS - 1 and m in (2, 7)):
                            pop_filler(99 if blk == 2 * PAIRS - 1 else blk)
                        for h in range(2):
                            head = 2 * pr + h
                            vtile = vt[m][head // 8][:, head % 8, :]
                            for qc in range(QC):
                                nc.tensor.matmul(
                                    av4[h][:, qc, :],
                                    pts[m][:, h, qc * P:(qc + 1) * P],
                                    vtile,
                                    start=(m == 0 and qc == 0),
                                    stop=(m == MT - 1 and qc == QC - 1),
                                    skip_group_check=True)
                    # evacuate each av bank with ONE full-tile copy (the
                    # full-bank read makes the next start=True bank-zero
                    # wait for it), then normalize from SBUF
                    ava = []
                    for h in range(2):
                        a = rp.tile([P, QC, D + 1], F32,
                                    name=f"ava{pr}_{n2}_{h}", tag="ava",
                                    bufs=4)
                        nc.vector.tensor_copy(a[:], av4[h][:])
                        ava.append(a)
                    aobs = []
                    for qc in range(QC):
                        aob = rp.tile([P, P], BF16, name=f"aob{pr}_{n2}_{qc}",
                                      tag="aob", bufs=8)
                        aobs.append(aob)
                        for h in range(2):
                            rec = rp.tile([P, 1], F32,
                                          name=f"rec{pr}_{n2}_{qc}_{h}",
                                          tag="rec", bufs=8)
                            nc.vector.reciprocal(rec[:],
                                                 ava[h][:, qc, D:D + 1])
                            nc.vector.tensor_scalar_mul(
                                aob[:, h * 64:(h + 1) * 64],
                                ava[h][:, qc, 0:D], rec[:, 0:1])
                    pending.append((aobs, pr, n2))
                    pass

            flush_pending()
            while fillers:
                fillers.popleft()[1]()
            nc.sync.dma_start(dbg_d.ap(), ao4t[:])
            nc.sync.dma_start(dbgq_d.ap(), qT[0][:])
            nc.sync.dma_start(dbgp_d.ap(), pts[0][:])
            for ot in range(MT):
                proj_chain(0, ot)
            for ot in range(MT):
                proj_chain(1, ot)

    nc.compile()
    return nc


def get_nc():
    if "nc" not in _cache:
        _cache["nc"] = _build()
    return _cache["nc"]


def _pair_group(a):
    """[C, cols] -> [128, 8, cols]: contraction tile ct = c // 128 moves
    to the middle dim (DoubleRow instruction g pairs tiles 2g, 2g+1)."""
    cols = a.shape[1]
    return np.ascontiguousarray(a.reshape(8, P, cols).transpose(1, 0, 2))


def _split_hi_lo(a):
    hi = a.astype(E4)
    lo = (a - hi.astype(np.float32)).astype(E5)
    return hi, lo


def kernel(x, w_qkv, w_proj, b_proj):
    x = np.asarray(x, dtype=np.float32)
    w_qkv = np.asarray(w_qkv, dtype=np.float32)
    w_proj = np.asarray(w_proj, dtype=np.float32)
    b_proj = np.asarray(b_proj, dtype=np.float32)

    wqh, wql = _split_hi_lo(np.ascontiguousarray(w_qkv.T))   # [C, 3C]
    w4 = _pair_group(wqh)
    w5 = _pair_group(wql)
    wph, wpl = _split_hi_lo(np.ascontiguousarray(w_proj.T))  # [C, C]
    wp4 = _pair_group(wph)
    wp5 = _pair_group(wpl)
    bias = np.ascontiguousarray(b_proj.reshape(MT, P).T).astype(np.float32)
    ident = np.eye(P, dtype=np.float32).astype(ml_dtypes.bfloat16)

    in_maps = []
    for b in range(N_CORES):
        xh, xl = _split_hi_lo(np.ascontiguousarray(x[b].T))  # [C, N]
        in_maps.append({
            "x4": _pair_group(xh),
            "x5": _pair_group(xl),
            "w4": w4, "w5": w5, "wp4": wp4, "wp5": wp5,
            "bias": bias, "ident": ident,
        })

    nc = get_nc()
    _cache["in_maps"] = in_maps
    res = bass_utils.run_bass_kernel_spmd(nc, in_maps,
                                          core_ids=list(range(N_CORES)))
    out = np.empty((B, N, C), dtype=np.float32)
    for b in range(N_CORES):
        out[b] = res.results[b]["outT"].T.astype(np.float32)
    return out
